# revision 1
# baseline (speedup 1.0000x reference)
"""Trainium2 Bass kernel for MergedColumnParallelLinearWithLoRA.

Computes  out = x @ W.T + concat(lora1(x), lora2(x))  where
lora_i(x)[t] = B_i[l_t] @ (A_i[l_t] @ x[t]) + bias_i[l_t],  l_t = indices[t].

Sharding: ROW-parallel (token-sharded) across 8 NeuronCores. Core c owns
tokens [c*1024, (c+1)*1024); x and indices are sharded along tokens, W /
lora weights are used in full by every core (streamed from HBM). This makes
the LoRA shrink naturally local (no replicated work, no collectives): each
core only computes s = A @ x_t for its own tokens.

Per-core device program:
  - x^T resident in SBUF ([128, 8, 16, 128], d-major tiles).
  - Augmented weight W_aug = [A1_flat; A2_flat; W] as 23 column-chunks of 512.
  - Chunk 0 = LoRA shrink: s1|s2 per token tile -> masked dispatch
    (s_masked = s * (lora_id_col == idx)), one-hot oh = (iota16 == idx),
    PE-transposed into resident s^T / oh^T tiles.
  - Chunks 1..22 = base GEMM, streamed W; the LoRA expand + bias
    ( y = [s_masked | oh] @ [B_flat_chunk; bias_chunk] ) accumulates into the
    same PSUM bank right after the 16 base k-matmuls (start=False).
  - All matmuls fp32r (full PE rate, ~1.4e-4 rel precision), N=512 uniform.
  - PSUM -> SBUF copies split across ScalarE/VectorE, then DMA out.
"""

import numpy as np

import concourse.bass as bass  # noqa: F401
import concourse.mybir as mybir
import concourse.tile as tile
from concourse import bacc
from concourse.masks import make_identity

T, D, O, L, R = 8192, 2048, 5632, 16, 16
NCORES = 8
TL = T // NCORES  # 1024 tokens per core
P = 128
KT = D // P  # 16 k-tiles
MTL = TL // P  # 8 local token tiles
SH = 2 * L * R  # 512 shrink columns (s1 | s2)
NF = 2 * O  # 11264 full output columns
NCH = NF // 512  # 22 base chunks
WA = SH + NF  # 11776 augmented columns = 23 chunks of 512
F32 = mybir.dt.float32
F32R = mybir.dt.float32r
I32 = mybir.dt.int32


def build_nc(reps=1, mode="full", bias_via="dma"):
    """mode: 'full' | 'base' (no LoRA shrink/expand).
    bias_via: 'dma' (indirect-DMA gather + vector add, assumes indices>=0)
              | 'pe' (one-hot K=16 matmul on the PE).
    """
    nc = bacc.Bacc("TRN2", target_bir_lowering=False, debug=False)

    xt = nc.dram_tensor("xt", [MTL, P, KT, P], F32, kind="ExternalInput")
    wt = nc.dram_tensor("wt", [NCH + 1, P, KT, 512], F32, kind="ExternalInput")
    b1 = nc.dram_tensor("b1", [2 * P + L, O], F32, kind="ExternalInput")
    b2 = nc.dram_tensor("b2", [2 * P + L, O], F32, kind="ExternalInput")
    c1 = nc.dram_tensor("c1", [L, O], F32, kind="ExternalInput")
    c2 = nc.dram_tensor("c2", [L, O], F32, kind="ExternalInput")
    idx = nc.dram_tensor("idx", [P, MTL], I32, kind="ExternalInput")
    out = nc.dram_tensor("out", [TL, NF], F32, kind="ExternalOutput")

    bdram = (b1, b2)
    cdram = (c1, c2)

    with tile.TileContext(nc) as tc:
        with (
            tc.tile_pool(name="const", bufs=1) as const,
            tc.tile_pool(name="wpool", bufs=2) as wpool,
            tc.tile_pool(name="bpool", bufs=2) as bpool,
            tc.tile_pool(name="spool", bufs=3) as spool,
            tc.tile_pool(name="opool", bufs=4) as opool,
            tc.tile_pool(name="gpool", bufs=4) as gpool,
            tc.tile_pool(name="ps_b", bufs=8, space="PSUM") as ps_b,
        ):
            # ---------------- resident constants ----------------
            # startup-critical DMAs first, k-sliced so the first shrink
            # matmuls only wait on their own k-slice
            t_xr = const.tile([P, MTL, KT, P], F32R, tag="xr", name="t_xr")
            t_w0 = wpool.tile([P, KT, 512], F32R, tag="w", name="t_w0")
            for kk in range(KT):
                nc.sync.dma_start(t_xr[:, 0, kk], xt[0, :, kk].bitcast(F32R))
                nc.sync.dma_start(t_w0[:, kk], wt[0, :, kk].bitcast(F32R))
            for mtl in range(1, MTL):
                nc.sync.dma_start(t_xr[:, mtl], xt[mtl].bitcast(F32R))

            t_idx = const.tile([P, MTL], I32, tag="idxi", name="t_idx")
            nc.sync.dma_start(t_idx[:], idx[:])
            t_idxf = const.tile([P, MTL], F32, tag="idxf", name="t_idxf")
            nc.vector.tensor_copy(t_idxf[:], t_idx[:])

            t_identf = const.tile([P, P], F32, tag="identf", name="t_identf")
            make_identity(nc, t_identf[:])
            t_ident = const.tile([P, P], F32R, tag="ident", name="t_ident")
            nc.vector.tensor_copy(t_ident[:], t_identf[:])

            # lora-id per shrink column: col j (within s1 or s2) -> j // R
            t_lidi = const.tile([P, 2, L, R], I32, tag="lidi", name="t_lidi")
            nc.gpsimd.iota(
                t_lidi[:], pattern=[[0, 2], [1, L], [0, R]], base=0, channel_multiplier=0
            )
            t_lid = const.tile([P, SH], F32, tag="lid", name="t_lid")
            nc.vector.tensor_copy(t_lid[:], t_lidi[:].rearrange("p a l r -> p (a l r)"))

            t_i16i = const.tile([P, L], I32, tag="i16i", name="t_i16i")
            nc.gpsimd.iota(t_i16i[:], pattern=[[1, L]], base=0, channel_multiplier=0)
            t_i16 = const.tile([P, L], F32, tag="i16", name="t_i16")
            nc.vector.tensor_copy(t_i16[:], t_i16i[:])

            # resident transposed masked-shrink + one-hot
            t_st = const.tile([P, MTL, 4 * P], F32R, tag="st", name="t_st")
            t_oh = const.tile([L, MTL, P], F32R, tag="oh", name="t_oh")

            for _rep in range(reps):
                # ---------------- chunk 0: LoRA shrink ----------------
                if mode == "full":
                    pend = []

                    def _transpose_sa(mtl, t_sa):
                        p_t = ps_b.tile([P, 4 * P], F32R, tag="b", name="p_t")
                        for j in range(4):
                            nc.tensor.transpose(
                                p_t[:, j * P : (j + 1) * P],
                                t_sa[:, j * P : (j + 1) * P],
                                t_ident[:],
                            )
                        nc.vector.tensor_copy(t_st[:, mtl, :], p_t[:])
                        if bias_via == "pe":
                            p_to = ps_b.tile([L, P], F32R, tag="b", name="p_to")
                            nc.tensor.transpose(
                                p_to[:], t_sa[:, SH : SH + L], t_ident[:]
                            )
                            nc.vector.tensor_copy(t_oh[:, mtl, :], p_to[:])

                    if _rep > 0:
                        t_w0 = wpool.tile([P, KT, 512], F32R, tag="w", name="t_w0")
                        nc.sync.dma_start(t_w0[:], wt[0].bitcast(F32R))
                    for mtl in range(MTL):
                        p_s = ps_b.tile([P, SH], F32, tag="b", name="p_s")
                        for kk in range(KT):
                            nc.tensor.matmul(
                                p_s[:],
                                t_xr[:, mtl, kk, :],
                                t_w0[:, kk, :],
                                start=(kk == 0),
                                stop=(kk == KT - 1),
                            )
                        idx_ap = t_idxf[:, mtl : mtl + 1]
                        t_sa = spool.tile([P, SH + L], F32R, tag="sa", name="t_sa")
                        nc.vector.scalar_tensor_tensor(
                            t_sa[:, 0:SH],
                            t_lid[:],
                            idx_ap,
                            p_s[:],
                            op0=mybir.AluOpType.is_equal,
                            op1=mybir.AluOpType.mult,
                        )
                        if bias_via == "pe":
                            nc.vector.tensor_scalar(
                                t_sa[:, SH : SH + L],
                                t_i16[:],
                                idx_ap,
                                None,
                                op0=mybir.AluOpType.is_equal,
                            )
                        pend.append((mtl, t_sa))
                        if len(pend) >= 2:
                            _transpose_sa(*pend.pop(0))
                    while pend:
                        _transpose_sa(*pend.pop(0))

                # ---------------- chunks 1..22: base + expand ----------------
                for ch in range(1, NCH + 1):
                    s, ci = divmod(ch - 1, NCH // 2)
                    t_wc = wpool.tile([P, KT, 512], F32R, tag="w", name="t_wc")
                    for kk in range(KT):
                        nc.sync.dma_start(t_wc[:, kk], wt[ch, :, kk].bitcast(F32R))
                    if mode == "full":
                        t_b = bpool.tile([P, 2, 512], F32R, tag="bb", name="t_b")
                        nc.sync.dma_start(
                            t_b[:],
                            bdram[s][0 : 2 * P, ci * 512 : (ci + 1) * 512]
                            .rearrange("(c p) o -> p c o", p=P)
                            .bitcast(F32R),
                        )
                        if bias_via == "pe":
                            t_bb = bpool.tile([L, 512], F32R, tag="bc", name="t_bb")
                            nc.sync.dma_start(
                                t_bb[:],
                                bdram[s][
                                    2 * P : 2 * P + L, ci * 512 : (ci + 1) * 512
                                ].bitcast(F32R),
                            )
                    for mtl in range(MTL):
                        p_b = ps_b.tile([P, 512], F32, tag="b", name="p_b")
                        do_exp = mode == "full"
                        for kk in range(KT):
                            nc.tensor.matmul(
                                p_b[:],
                                t_xr[:, mtl, kk, :],
                                t_wc[:, kk, :],
                                start=(kk == 0),
                                stop=(not do_exp and kk == KT - 1),
                            )
                        if do_exp:
                            for c in range(2):
                                nc.tensor.matmul(
                                    p_b[:],
                                    t_st[:, mtl, (2 * s + c) * P : (2 * s + c + 1) * P],
                                    t_b[:, c, :],
                                    start=False,
                                    stop=(bias_via != "pe" and c == 1),
                                )
                            if bias_via == "pe":
                                nc.tensor.matmul(
                                    p_b[:],
                                    t_oh[:, mtl, :],
                                    t_bb[:],
                                    start=False,
                                    stop=True,
                                )
                        t_out = opool.tile([P, 512], F32, tag="o", name="t_out")
                        if do_exp and bias_via == "dma":
                            t_bg = gpool.tile([P, 512], F32, tag="g", name="t_bg")
                            nc.gpsimd.indirect_dma_start(
                                out=t_bg[:],
                                out_offset=None,
                                in_=cdram[s][:],
                                in_offset=bass.IndirectOffsetOnAxis(
                                    ap=t_idx[:, mtl : mtl + 1], axis=0
                                ),
                                element_offset=ci * 512,
                            )
                            nc.vector.tensor_tensor(
                                t_out[:], p_b[:], t_bg[:], op=mybir.AluOpType.add
                            )
                        elif (ch + mtl) % 2 == 0:
                            nc.vector.tensor_copy(t_out[:], p_b[:])
                        else:
                            nc.scalar.copy(t_out[:], p_b[:])
                        nc.sync.dma_start(
                            out[
                                mtl * P : (mtl + 1) * P,
                                (ch - 1) * 512 : ch * 512,
                            ],
                            t_out[:],
                        )

    nc.compile()
    return nc


# ---------------------------------------------------------------------------
# host-side sharding / unsharding
# ---------------------------------------------------------------------------


def shard_inputs(x, W, lora_a1, lora_a2, lora_b1, lora_b2, bias1, bias2, indices):
    x = np.asarray(x, np.float32)
    W = np.asarray(W, np.float32)
    indices = np.asarray(indices, np.int32)

    a1f = np.asarray(lora_a1, np.float32).reshape(L * R, D)
    a2f = np.asarray(lora_a2, np.float32).reshape(L * R, D)
    w_aug = np.concatenate([a1f, a2f, W], axis=0)  # [11776, 2048]
    # wt[ch, p, kk, j] = w_aug[ch*512 + j, kk*128 + p]
    wt = np.ascontiguousarray(
        w_aug.T.reshape(KT, P, NCH + 1, 512).transpose(2, 1, 0, 3)
    )

    def bmat(lb, bias):
        bf = np.asarray(lb, np.float32).transpose(0, 2, 1).reshape(L * R, O)
        return np.ascontiguousarray(
            np.concatenate([bf, np.asarray(bias, np.float32)], axis=0)
        )

    b1m = bmat(lora_b1, bias1)
    b2m = bmat(lora_b2, bias2)

    # xt[c][mtl, p, kk, m] = x[c*1024 + mtl*128 + m, kk*128 + p]
    xts = x.reshape(NCORES, MTL, P, KT, P).transpose(0, 1, 4, 3, 2)
    idxs = indices.reshape(NCORES, MTL, P).transpose(0, 2, 1)

    in_maps = []
    for c in range(NCORES):
        in_maps.append(
            {
                "xt": np.ascontiguousarray(xts[c]),
                "wt": wt,
                "b1": b1m,
                "b2": b2m,
                "c1": np.ascontiguousarray(np.asarray(bias1, np.float32)),
                "c2": np.ascontiguousarray(np.asarray(bias2, np.float32)),
                "idx": np.ascontiguousarray(idxs[c]),
            }
        )
    return in_maps


def unshard_output(results):
    out = np.empty((T, NF), np.float32)
    for c in range(NCORES):
        out[c * TL : (c + 1) * TL, :] = results[c]["out"]
    return out


_CACHE = {}


def get_nc():
    if "nc" not in _CACHE:
        _CACHE["nc"] = build_nc()
    return _CACHE["nc"]


def kernel(**inputs):
    from concourse import bass2jax

    nc = get_nc()
    in_maps = shard_inputs(**inputs)
    results = bass2jax.run_bass_via_pjrt(nc, in_maps, n_cores=NCORES)
    return unshard_output(results)



# revision 2
# speedup vs baseline: 1.0258x; 1.0258x over previous
"""Trainium2 Bass kernel for MergedColumnParallelLinearWithLoRA.

Computes  out = x @ W.T + concat(lora1(x), lora2(x))  where
lora_i(x)[t] = B_i[l_t] @ (A_i[l_t] @ x[t]) + bias_i[l_t],  l_t = indices[t].

Sharding: ROW-parallel (token-sharded) across 8 NeuronCores, with tokens
globally SORTED by lora id on the host. Core c owns 1024 consecutive sorted
tokens; each 128-token tile then spans a tiny contiguous lora window
(<= WLOR loras, typically 1-2 for uniform routing). W is streamed in full by
every core; no collectives.

Per-core device program (all matmul operands bf16, fp32 PSUM accumulate):
  - x^T resident in SBUF ([128, 8, 16, 128] d-major tiles), bf16.
  - Phase A (shrink): per tile/slice, s = A_window @ x_t^T directly in
    transposed form ([64 coords, 128 tokens] PSUM; A-window stationary).
    Masked dispatch + bias indicators come from a host-built 0/1 mask:
      st[0:64]  = s * mask        (coords of lora wg+j zeroed unless idx==wg+j)
      st[64:68] = mask indicators ((idx == wg+j); multiplies the bias row)
  - Phase B: 22 output chunks of 512. Per (chunk, tile): 16 streamed base
    matmuls (x^T stationary, W moving) + ONE fused LoRA-expand+bias matmul
    (st stationary K=68, [B_rows; bias_rows] window moving) into the same
    PSUM bank (start=False). No bias gather, no one-hot matmuls.
  - PSUM -> SBUF copies split across ScalarE/VectorE (fp32 -> bf16), DMA out
    in bf16; host upcasts and un-permutes rows.

Rel error ~2.5e-3 (bf16 operands + bf16 output rounding), ~8x inside the
2e-2 gate.
"""

import numpy as np
import ml_dtypes

import concourse.bass as bass  # noqa: F401
import concourse.mybir as mybir
import concourse.tile as tile
from concourse import bacc

T, D, O, L, R = 8192, 2048, 5632, 16, 16
NCORES = 8
TL = T // NCORES  # 1024 tokens per core
P = 128
KT = D // P  # 16 k-tiles
MTL = TL // P  # 8 local token tiles
NF = 2 * O  # 11264 full output columns
NCH = NF // 512  # 22 chunks (11 per slice)
NCS = NCH // 2  # 11 chunks per slice

WLOR = 4  # loras per tile window
SC = WLOR * R  # 64 shrink coords per slice per tile
SB = SC + WLOR  # 68 = coords + bias-indicator rows

F32 = mybir.dt.float32
BF16 = mybir.dt.bfloat16
BF = ml_dtypes.bfloat16


def build_nc(reps=1):
    nc = bacc.Bacc("TRN2", target_bir_lowering=False, debug=False)

    xt = nc.dram_tensor("xt", [MTL, P, KT, P], BF16, kind="ExternalInput")
    wt = nc.dram_tensor("wt", [NCH, P, KT, 512], BF16, kind="ExternalInput")
    aw = nc.dram_tensor("aw", [P, MTL, 2, KT, SC], BF16, kind="ExternalInput")
    bw = nc.dram_tensor("bw", [2, SB, MTL, O], BF16, kind="ExternalInput")
    mm = nc.dram_tensor("mm", [SB, MTL, P], BF16, kind="ExternalInput")
    out = nc.dram_tensor("out", [TL, NF], BF16, kind="ExternalOutput")

    with tile.TileContext(nc) as tc:
        with (
            tc.tile_pool(name="const", bufs=1) as const,
            tc.tile_pool(name="awpool", bufs=1) as awpool,
            tc.tile_pool(name="stpool", bufs=2) as stpool,
            tc.tile_pool(name="wpool", bufs=2) as wpool,
            tc.tile_pool(name="bwpool", bufs=2) as bwpool,
            tc.tile_pool(name="opool", bufs=4) as opool,
            tc.tile_pool(name="ps", bufs=8, space="PSUM") as ps,
        ):
            # ---------------- resident constants ----------------
            t_xr = const.tile([P, MTL, KT, P], BF16, tag="xr", name="t_xr")
            for kk in range(KT):
                nc.sync.dma_start(t_xr[:, 0, kk], xt[0, :, kk])
            for mtl in range(1, MTL):
                nc.sync.dma_start(t_xr[:, mtl], xt[mtl])
            t_mm = const.tile([SB, MTL, P], BF16, tag="mm", name="t_mm")
            nc.sync.dma_start(t_mm[:], mm[:])

            for _rep in range(reps):
                # ---------------- phase A: LoRA shrink ----------------
                t_aw = awpool.tile([P, MTL, 2, KT, SC], BF16, tag="aw", name="t_aw")
                for mtl in range(MTL):
                    nc.sync.dma_start(t_aw[:, mtl], aw[:, mtl])
                t_st = stpool.tile([SB, 2, MTL, P], BF16, tag="st", name="t_st")
                for mtl in range(MTL):
                    for s in range(2):
                        p_s = ps.tile([SC, P], F32, tag="b", name="p_s")
                        for kk in range(KT):
                            nc.tensor.matmul(
                                p_s[:],
                                t_aw[:, mtl, s, kk, :],
                                t_xr[:, mtl, kk, :],
                                start=(kk == 0),
                                stop=(kk == KT - 1),
                            )
                        nc.vector.tensor_tensor(
                            t_st[0:SC, s, mtl, :],
                            p_s[:],
                            t_mm[0:SC, mtl, :],
                            op=mybir.AluOpType.mult,
                        )
                        nc.scalar.copy(
                            t_st[SC:SB, s, mtl, :], t_mm[SC:SB, mtl, :]
                        )

                # ---------------- phase B: base + fused expand ----------------
                for ch in range(NCH):
                    s, ci = divmod(ch, NCS)
                    t_wc = wpool.tile([P, KT, 512], BF16, tag="w", name="t_wc")
                    for kk in range(KT):
                        nc.sync.dma_start(t_wc[:, kk], wt[ch, :, kk])
                    t_bw = bwpool.tile([SB, MTL, 512], BF16, tag="bw", name="t_bw")
                    nc.sync.dma_start(
                        t_bw[:], bw[s, :, :, ci * 512 : (ci + 1) * 512]
                    )
                    for mtl in range(MTL):
                        p_b = ps.tile([P, 512], F32, tag="b", name="p_b")
                        for kk in range(KT):
                            nc.tensor.matmul(
                                p_b[:],
                                t_xr[:, mtl, kk, :],
                                t_wc[:, kk, :],
                                start=(kk == 0),
                                stop=False,
                            )
                        nc.tensor.matmul(
                            p_b[:],
                            t_st[:, s, mtl, :],
                            t_bw[:, mtl, :],
                            start=False,
                            stop=True,
                        )
                        t_out = opool.tile([P, 512], BF16, tag="o", name="t_out")
                        if (ch + mtl) % 2 == 0:
                            nc.vector.tensor_copy(t_out[:], p_b[:])
                        else:
                            nc.scalar.copy(t_out[:], p_b[:])
                        nc.sync.dma_start(
                            out[
                                mtl * P : (mtl + 1) * P,
                                ch * 512 : (ch + 1) * 512,
                            ],
                            t_out[:],
                        )

    nc.compile()
    return nc


# ---------------------------------------------------------------------------
# host-side sharding / unsharding
# ---------------------------------------------------------------------------


def _prep(x, W, lora_a1, lora_a2, lora_b1, lora_b2, bias1, bias2, indices):
    x = np.asarray(x, np.float32)
    W = np.asarray(W, np.float32)
    indices = np.asarray(indices, np.int32)

    perm = np.argsort(indices, kind="stable")
    idx_s = indices[perm]
    x_s = x[perm]

    # wt[ch, p, kk, j] = W[ch*512 + j, kk*128 + p]
    wt = np.ascontiguousarray(
        W.T.reshape(KT, P, NCH, 512).transpose(2, 1, 0, 3)
    ).astype(BF)

    # A_s as [d, lora, r]
    A = [
        np.asarray(lora_a1, np.float32).transpose(2, 0, 1),
        np.asarray(lora_a2, np.float32).transpose(2, 0, 1),
    ]
    B = [np.asarray(lora_b1, np.float32), np.asarray(lora_b2, np.float32)]
    bias = [np.asarray(bias1, np.float32), np.asarray(bias2, np.float32)]

    # xt[c][t, p, kk, m] = x_s[c*1024 + t*128 + m, kk*128 + p]
    xts = x_s.reshape(NCORES, MTL, P, KT, P).transpose(0, 1, 4, 3, 2).astype(BF)

    in_maps = []
    for c in range(NCORES):
        idx_c = idx_s[c * TL : (c + 1) * TL].reshape(MTL, P)
        g_lo = idx_c.min(axis=1)
        g_hi = idx_c.max(axis=1)
        span = (g_hi - g_lo + 1).max()
        if span > WLOR:
            raise ValueError(
                f"tile lora span {span} exceeds window {WLOR}; "
                "routing distribution too skewed for compiled window size"
            )
        wg = np.minimum(g_lo, L - WLOR)  # [MTL] window starts

        awc = np.empty((P, MTL, 2, KT, SC), np.float32)
        bwc = np.zeros((2, SB, MTL, O), np.float32)
        mmc = np.zeros((SB, MTL, P), np.float32)
        for t in range(MTL):
            w0 = int(wg[t])
            for s in range(2):
                # aw[p, t, s, kk, c] = A_s[kk*128+p, w0 + c//R, c%R]
                Awin = A[s][:, w0 : w0 + WLOR, :].reshape(D, SC)
                awc[:, t, s] = Awin.reshape(KT, P, SC).transpose(1, 0, 2)
                # bw rows: q<SC -> B_s[w0+q//R, :, q%R]; q>=SC -> bias rows
                bwc[s, 0:SC, t] = (
                    B[s][w0 : w0 + WLOR].transpose(0, 2, 1).reshape(SC, O)
                )
                bwc[s, SC:SB, t] = bias[s][w0 : w0 + WLOR]
            # mask rows: indicator (idx == w0 + j) per coord / bias row
            lid = np.concatenate(
                [np.arange(WLOR).repeat(R), np.arange(WLOR)]
            )  # [SB]
            mmc[:, t, :] = (idx_c[t][None, :] == (w0 + lid)[:, None]).astype(
                np.float32
            )

        in_maps.append(
            {
                "xt": np.ascontiguousarray(xts[c]),
                "wt": wt,
                "aw": np.ascontiguousarray(awc.astype(BF)),
                "bw": np.ascontiguousarray(bwc.astype(BF)),
                "mm": np.ascontiguousarray(mmc.astype(BF)),
            }
        )
    return in_maps, perm


def shard_inputs(**inputs):
    return _prep(**inputs)[0]


def unshard_output(results, perm):
    out = np.empty((T, NF), np.float32)
    sorted_out = np.concatenate(
        [np.asarray(results[c]["out"], dtype=BF) for c in range(NCORES)], axis=0
    ).astype(np.float32)
    out[perm] = sorted_out
    return out


_CACHE = {}


def get_nc():
    if "nc" not in _CACHE:
        _CACHE["nc"] = build_nc()
    return _CACHE["nc"]


def kernel(**inputs):
    from concourse import bass2jax

    nc = get_nc()
    in_maps, perm = _prep(**inputs)
    results = bass2jax.run_bass_via_pjrt(nc, in_maps, n_cores=NCORES)
    return unshard_output(results, perm)
